# revision 23
# baseline (speedup 1.0000x reference)
"""Cross-attention kernel for 8 Trainium2 NeuronCores.

Sharding: data-parallel over batch (B=2) x tensor-parallel over heads
(16 heads -> 4 groups of 4 heads).  Core c handles batch c//4, head
group c%4.  Each core computes, for its 4 heads:
    Q^T = Wq_g^T x_b^T        [256, 2048]   (d-on-partitions layout)
    K^T = Wk_g^T y_b^T        [256, 2048]
    V   = y_b Wv_g            [2048, 256]   (n-on-partitions layout)
    S^T_h = K_h Q_h^T / 8; P^T = exp(S^T)
    O^T_h (+row sums via a ones-column in V) = [V_h|1]^T P^T
    partial = (O^T/rowsum)^T Wp_g           [2048, 1024]
The 4 partials per batch are summed on the host and bp is added.

Matmuls run as float32r (full-rate fp32 on the PE at moving-dim>=256).
Head pairs share the PE array via tile_position row packing for the
S^T matmuls; two query blocks are interleaved per head pair so the
exp (ScalarE) chain of one block hides the semaphore latency of the
other.
"""

import numpy as np

B = 2
N = 2048          # query sequence length
M = 2048          # key sequence length
DIM = 1024
HEAD_DIM = 64
SCALE = HEAD_DIM ** -0.5
NCORES = 8
GH = 4            # heads per core
J = GH * HEAD_DIM # 256 projected columns per core
KC = DIM // 128   # 8 contraction chunks
NT = M // 128     # 16 key tiles
IBS = 512         # i-block size
IB = N // IBS     # 4 i-blocks

PACK_S = True

_NC = None


def _build():
    from contextlib import ExitStack

    import concourse.bass as bass
    import concourse.tile as tile
    from concourse import bacc, mybir
    from concourse.bass import ts, ds
    from concourse.masks import make_identity

    f32 = mybir.dt.float32
    f32r = mybir.dt.float32r
    Exp = mybir.ActivationFunctionType.Exp

    nc = bacc.Bacc("TRN2", target_bir_lowering=False, debug=False,
                   num_devices=NCORES)
    xT = nc.dram_tensor("xT", [DIM, N], f32r, kind="ExternalInput").ap()
    yT = nc.dram_tensor("yT", [DIM, M], f32r, kind="ExternalInput").ap()
    wq = nc.dram_tensor("wq", [DIM, J], f32r, kind="ExternalInput").ap()
    wk = nc.dram_tensor("wk", [DIM, J], f32r, kind="ExternalInput").ap()
    wv = nc.dram_tensor("wv", [DIM, J], f32r, kind="ExternalInput").ap()
    wp = nc.dram_tensor("wp", [J, DIM], f32r, kind="ExternalInput").ap()
    out = nc.dram_tensor("out", [N, DIM], f32, kind="ExternalOutput").ap()

    with tile.TileContext(nc) as tc, ExitStack() as top:
        wpool = top.enter_context(tc.tile_pool(name="weights", bufs=1))
        wq_sb = wpool.tile([128, KC, J], f32r, name="wq_sb")
        wk_sb = wpool.tile([128, KC, J], f32r, name="wk_sb")
        wv_sb = wpool.tile([128, KC, J], f32r, name="wv_sb")
        wp_sb = wpool.tile([128, 2, DIM], f32r, name="wp_sb")
        # wq + x stream on the SP (sync) HWDGE queue; everything else on
        # the Activation HWDGE queue so Q's inputs aren't stuck behind 12MB
        wq_r = wq.rearrange("(c p) j -> p c j", p=128)
        nc.sync.dma_start(wq_sb[:, 0, :], wq_r[:, 0, :])
        nc.scalar.dma_start(wk_sb, wk.rearrange("(c p) j -> p c j", p=128))
        nc.scalar.dma_start(wv_sb, wv.rearrange("(c p) j -> p c j", p=128))
        nc.scalar.dma_start(wp_sb, wp.rearrange("(t p) c -> p t c", p=128))

        big = top.enter_context(tc.tile_pool(name="big", bufs=1))
        QT = [big.tile([128, N], f32r, name=f"qt{t}") for t in range(2)]
        KT = [big.tile([128, M], f32r, name=f"kt{t}") for t in range(2)]
        V_sb = big.tile([128, NT, GH, HEAD_DIM + 1], f32r, name="v_sb")
        # ones column for the row-sum trick: fill everything with 1.0 once;
        # the V evacuation below overwrites columns 0..63 of each (n, h)
        nc.vector.memset(V_sb.bitcast(f32), 1.0)

        ot_tiles = {}
        otpool = top.enter_context(tc.tile_pool(name="otpool", bufs=8))

        # ---- projections -------------------------------------------------
        with tc.tile_pool(name="ystream", bufs=1) as ypool:
            yt = ypool.tile([128, KC, M], f32r, name="yt")
            for c in range(KC):
                nc.scalar.dma_start(yt[:, c, :], yT[ts(c, 128), :])

            # Q^T: stream xT, all 8 psum banks accumulate over k-chunks
            with tc.tile_pool(name="xstream", bufs=3) as xpool, \
                 tc.tile_pool(name="qpsum", bufs=1, space="PSUM") as qpsum:
                qps = [qpsum.tile([128, 512], f32, name=f"qps{t}")
                       for t in range(8)]
                for c in range(KC):
                    xt = xpool.tile([128, N], f32r, name="xt")
                    nc.sync.dma_start(xt, xT[ts(c, 128), :])
                    if c == 0:
                        nc.sync.dma_start(wq_sb[:, 1:KC, :], wq_r[:, 1:KC, :])
                    for jt in range(2):
                        for ic in range(4):
                            nc.tensor.matmul(
                                qps[jt * 4 + ic],
                                wq_sb[:, c, ts(jt, 128)],
                                xt[:, ts(ic, 512)],
                                start=(c == 0), stop=(c == KC - 1))
                for jt in range(2):
                    for ic in range(4):
                        nc.vector.tensor_copy(QT[jt][:, ts(ic, 512)],
                                              qps[jt * 4 + ic])

            # K^T from resident yT
            with tc.tile_pool(name="kpsum", bufs=1, space="PSUM") as kpsum:
                kps = [kpsum.tile([128, 512], f32, name=f"kps{t}")
                       for t in range(8)]
                for c in range(KC):
                    for jt in range(2):
                        for ic in range(4):
                            nc.tensor.matmul(
                                kps[jt * 4 + ic],
                                wk_sb[:, c, ts(jt, 128)],
                                yt[:, c, ts(ic, 512)],
                                start=(c == 0), stop=(c == KC - 1))
                for jt in range(2):
                    for ic in range(4):
                        nc.vector.tensor_copy(KT[jt][:, ts(ic, 512)],
                                              kps[jt * 4 + ic])

            # V^T = Wv_g^T y^T with N=512 moving (full-rate), then
            # PE-transpose 128x128 tiles into the [n, j] layout
            with tc.tile_pool(name="vtsb", bufs=1) as vtpool:
                vt_sb = vtpool.tile([128, 2, M], f32, name="vt_sb")
                with tc.tile_pool(name="vtpsum", bufs=1,
                                  space="PSUM") as vtpsum:
                    vtps = [vtpsum.tile([128, 512], f32, name=f"vtps{t}")
                            for t in range(8)]
                    for c in range(KC):
                        for jt in range(2):
                            for nch in range(4):
                                nc.tensor.matmul(
                                    vtps[jt * 4 + nch],
                                    wv_sb[:, c, ts(jt, 128)],
                                    yt[:, c, ts(nch, 512)],
                                    start=(c == 0), stop=(c == KC - 1))
                    for jt in range(2):
                        for nch in range(4):
                            nc.vector.tensor_copy(vt_sb[:, jt, ts(nch, 512)],
                                                  vtps[jt * 4 + nch])
                with tc.tile_pool(name="idpool", bufs=1) as idpool, \
                     tc.tile_pool(name="trpsum", bufs=4,
                                  space="PSUM") as trpsum:
                    ident = idpool.tile([128, 128], f32, name="ident")
                    make_identity(nc, ident)
                    for jt in range(2):
                        for n in range(NT):
                            trp = trpsum.tile([128, 128], f32, name="trp")
                            nc.tensor.transpose(
                                trp, vt_sb[:, jt, ts(n, 128)], ident)
                            nc.vector.tensor_copy(
                                V_sb[:, n, 2 * jt:2 * jt + 2, 0:HEAD_DIM],
                                trp.rearrange("p (h d) -> p h d", h=2))

        # ---- attention ---------------------------------------------------
        with tc.tile_pool(name="ppool", bufs=4) as ppool, \
             tc.tile_pool(name="rpool", bufs=4) as rpool, \
             tc.tile_pool(name="rbpool", bufs=3) as rbpool, \
             tc.tile_pool(name="rdram", bufs=3, space="DRAM") as rdram, \
             tc.tile_pool(name="spsum", bufs=2, space="PSUM") as spsum, \
             tc.tile_pool(name="opsum", bufs=1, space="PSUM") as opsum:
            for pr in range(2):
                for ibp in range(2):
                    blocks = [2 * ibp, 2 * ibp + 1]
                    oacc = {}
                    for ab in range(2):
                        for lh in range(2):
                            oacc[(ab, lh)] = opsum.tile(
                                [HEAD_DIM + 1, IBS], f32, name=f"o{ab}{lh}")
                    pts = {}
                    for n in range(NT):
                        # S^T + exp for both blocks of this n
                        for ab, ib in enumerate(blocks):
                            i_sl = ts(ib, IBS)
                            sp = spsum.tile([128, 2 * IBS], f32, name="sp")
                            tp_lo = dict(tile_position=(0, 0)) if PACK_S else {}
                            tp_hi = dict(tile_position=(64, 0)) if PACK_S else {}
                            nc.tensor.matmul(
                                sp[:, 0:IBS],
                                KT[pr][0:64, ts(n, 128)],
                                QT[pr][0:64, i_sl],
                                start=True, stop=True, **tp_lo)
                            nc.tensor.matmul(
                                sp[:, IBS:2 * IBS],
                                KT[pr][64:128, ts(n, 128)],
                                QT[pr][64:128, i_sl],
                                start=True, stop=True, **tp_hi)
                            pt = ppool.tile([128, 2 * IBS], f32r, name="pt")
                            nc.scalar.activation(pt, sp, Exp, bias=0.0,
                                                 scale=float(SCALE))
                            pts[(ab, n)] = pt
                        # O^T accumulation for the previous n (software
                        # pipeline: keeps PE from blocking on the fresh exp)
                        if n > 0:
                            for ab in range(2):
                                pt = pts.pop((ab, n - 1))
                                for lh in range(2):
                                    nc.tensor.matmul(
                                        oacc[(ab, lh)],
                                        V_sb[:, n - 1, 2 * pr + lh, :],
                                        pt[:, lh * IBS:(lh + 1) * IBS],
                                        start=(n - 1 == 0), stop=False)
                    for ab in range(2):
                        pt = pts.pop((ab, NT - 1))
                        for lh in range(2):
                            nc.tensor.matmul(
                                oacc[(ab, lh)],
                                V_sb[:, NT - 1, 2 * pr + lh, :],
                                pt[:, lh * IBS:(lh + 1) * IBS],
                                start=False, stop=True)
                    # evacuate + normalize both blocks
                    # PSUM-releasing evacuations for BOTH blocks first (the
                    # next round's O matmuls wait on these slots), then the
                    # off-critical-path normalize chains.
                    blk = {}
                    for ab, ib in enumerate(blocks):
                        o_lo, o_hi = oacc[(ab, 0)], oacc[(ab, 1)]
                        ot = otpool.tile([128, IBS], f32r, name="ot")
                        nc.vector.tensor_copy(ot[0:64, :], o_lo[0:64, :])
                        nc.vector.tensor_copy(ot[64:128, :], o_hi[0:64, :])
                        rs_lo = rpool.tile([1, IBS], f32, name="rslo")
                        rs_hi = rpool.tile([1, IBS], f32, name="rshi")
                        nc.vector.tensor_copy(rs_lo, o_lo[64:65, :])
                        nc.vector.tensor_copy(rs_hi, o_hi[64:65, :])
                        blk[ab] = (ot, rs_lo, rs_hi)
                    for ab, ib in enumerate(blocks):
                        ot, rs_lo, rs_hi = blk[ab]
                        rd = rdram.tile([2, IBS], f32, name="rd")
                        nc.sync.dma_start(rd[0:1, :], rs_lo)
                        nc.sync.dma_start(rd[1:2, :], rs_hi)
                        rb = rbpool.tile([128, IBS], f32, name="rb")
                        nc.sync.dma_start(rb[0:64, :],
                                          rd[0:1, :].partition_broadcast(64))
                        nc.sync.dma_start(rb[64:128, :],
                                          rd[1:2, :].partition_broadcast(64))
                        rb2 = rbpool.tile([128, IBS], f32, name="rb2")
                        nc.vector.reciprocal(rb2, rb)
                        nc.vector.tensor_mul(ot, ot.bitcast(f32), rb2)
                        ot_tiles[(ib, pr)] = ot

        # ---- output projection ------------------------------------------
        with tc.tile_pool(name="obpool", bufs=4) as obpool, \
             tc.tile_pool(name="oppsum", bufs=4, space="PSUM") as oppsum:
            for ib in range(IB):
                for icr in range(IBS // 128):
                    for cc in range(DIM // 512):
                        op = oppsum.tile([128, 512], f32, name="op")
                        for jt in range(2):
                            nc.tensor.matmul(
                                op,
                                ot_tiles[(ib, jt)][:, ts(icr, 128)],
                                wp_sb[:, jt, ts(cc, 512)],
                                start=(jt == 0), stop=(jt == 1))
                        ob = obpool.tile([128, 512], f32, name="ob")
                        nc.vector.tensor_copy(ob, op)
                        nc.sync.dma_start(
                            out[ds(ib * IBS + icr * 128, 128), ts(cc, 512)],
                            ob)

    nc.compile()
    return nc


def _get_nc():
    global _NC
    if _NC is None:
        _NC = _build()
    return _NC


def _shard_inputs(x, y, Wq, Wk, Wv, Wp):
    x = np.asarray(x, np.float32)
    y = np.asarray(y, np.float32)
    Wq = np.asarray(Wq, np.float32)
    Wk = np.asarray(Wk, np.float32)
    Wv = np.asarray(Wv, np.float32)
    Wp = np.asarray(Wp, np.float32)
    xT = [np.ascontiguousarray(x[b].T) for b in range(B)]
    yT = [np.ascontiguousarray(y[b].T) for b in range(B)]
    in_maps = []
    for c in range(NCORES):
        b, g = divmod(c, NCORES // B)
        sl = slice(g * J, (g + 1) * J)
        in_maps.append({
            "xT": xT[b],
            "yT": yT[b],
            "wq": np.ascontiguousarray(Wq[:, sl]),
            "wk": np.ascontiguousarray(Wk[:, sl]),
            "wv": np.ascontiguousarray(Wv[:, sl]),
            "wp": np.ascontiguousarray(Wp[sl, :]),
        })
    return in_maps


def run(inputs, trace=False, **spmd_kwargs):
    from concourse.bass_utils import run_bass_kernel_spmd
    nc = _get_nc()
    in_maps = _shard_inputs(inputs["x"], inputs["y"], inputs["Wq"],
                            inputs["Wk"], inputs["Wv"], inputs["Wp"])
    res = run_bass_kernel_spmd(nc, in_maps, core_ids=list(range(NCORES)),
                               trace=trace, **spmd_kwargs)
    bp = np.asarray(inputs["bp"], np.float32)
    gpb = NCORES // B
    full = np.empty((B, N, DIM), np.float32)
    for b in range(B):
        acc = res.results[b * gpb]["out"].astype(np.float32)
        for g in range(1, gpb):
            acc = acc + res.results[b * gpb + g]["out"]
        full[b] = acc + bp
    return full, res


def kernel(**inputs):
    out, _ = run(inputs, trace=False)
    return out


# revision 25
# speedup vs baseline: 1.0175x; 1.0175x over previous
"""Cross-attention kernel for 8 Trainium2 NeuronCores.

Sharding: data-parallel over batch (B=2) x tensor-parallel over heads
(16 heads -> 4 groups of 4 heads).  Core c handles batch c//4, head
group c%4.  Each core computes, for its 4 heads:
    Q^T = Wq_g^T x_b^T        [256, 2048]   (d-on-partitions layout)
    K^T = Wk_g^T y_b^T        [256, 2048]
    V   = y_b Wv_g            [2048, 256]   (n-on-partitions layout)
    S^T_h = K_h Q_h^T / 8; P^T = exp(S^T)
    O^T_h (+row sums via a ones-column in V) = [V_h|1]^T P^T
    partial = (O^T/rowsum)^T Wp_g           [2048, 1024]
The 4 partials per batch are summed on the host and bp is added.

Matmuls run as float32r (full-rate fp32 on the PE at moving-dim>=256).
Head pairs share the PE array via tile_position row packing for the
S^T matmuls; two query blocks are interleaved per head pair so the
exp (ScalarE) chain of one block hides the semaphore latency of the
other.
"""

import numpy as np

B = 2
N = 2048          # query sequence length
M = 2048          # key sequence length
DIM = 1024
HEAD_DIM = 64
SCALE = HEAD_DIM ** -0.5
NCORES = 8
GH = 4            # heads per core
J = GH * HEAD_DIM # 256 projected columns per core
KC = DIM // 128   # 8 contraction chunks
NT = M // 128     # 16 key tiles
IBS = 512         # i-block size
IB = N // IBS     # 4 i-blocks

PACK_S = True

_NC = None


def _build():
    from contextlib import ExitStack

    import concourse.bass as bass
    import concourse.tile as tile
    from concourse import bacc, mybir
    from concourse.bass import ts, ds
    from concourse.masks import make_identity

    f32 = mybir.dt.float32
    f32r = mybir.dt.float32r
    Exp = mybir.ActivationFunctionType.Exp

    nc = bacc.Bacc("TRN2", target_bir_lowering=False, debug=False,
                   num_devices=NCORES)
    xT = nc.dram_tensor("xT", [DIM, N], f32r, kind="ExternalInput").ap()
    yT = nc.dram_tensor("yT", [DIM, M], f32r, kind="ExternalInput").ap()
    wq = nc.dram_tensor("wq", [DIM, J], f32r, kind="ExternalInput").ap()
    wk = nc.dram_tensor("wk", [DIM, J], f32r, kind="ExternalInput").ap()
    wv = nc.dram_tensor("wv", [DIM, J], f32r, kind="ExternalInput").ap()
    wp = nc.dram_tensor("wp", [J, DIM], f32r, kind="ExternalInput").ap()
    out = nc.dram_tensor("out", [N, DIM], f32, kind="ExternalOutput").ap()

    with tile.TileContext(nc) as tc, ExitStack() as top:
        wpool = top.enter_context(tc.tile_pool(name="weights", bufs=1))
        wq_sb = wpool.tile([128, KC, J], f32r, name="wq_sb")
        wk_sb = wpool.tile([128, KC, J], f32r, name="wk_sb")
        wv_sb = wpool.tile([128, KC, J], f32r, name="wv_sb")
        wp_sb = wpool.tile([128, 2, DIM], f32r, name="wp_sb")
        # wq + x stream on the SP (sync) HWDGE queue; everything else on
        # the Activation HWDGE queue so Q's inputs aren't stuck behind 12MB
        wq_r = wq.rearrange("(c p) j -> p c j", p=128)
        nc.sync.dma_start(wq_sb[:, 0, :], wq_r[:, 0, :])
        nc.scalar.dma_start(wk_sb, wk.rearrange("(c p) j -> p c j", p=128))
        nc.scalar.dma_start(wv_sb, wv.rearrange("(c p) j -> p c j", p=128))
        nc.scalar.dma_start(wp_sb, wp.rearrange("(t p) c -> p t c", p=128))

        big = top.enter_context(tc.tile_pool(name="big", bufs=1))
        QT = [big.tile([128, N], f32r, name=f"qt{t}") for t in range(2)]
        KT = [big.tile([128, M], f32r, name=f"kt{t}") for t in range(2)]
        V_sb = big.tile([128, NT, GH, HEAD_DIM + 1], f32r, name="v_sb")
        # ones column for the row-sum trick: fill everything with 1.0 once;
        # the V evacuation below overwrites columns 0..63 of each (n, h)
        nc.vector.memset(V_sb.bitcast(f32), 1.0)

        ot_tiles = {}
        otpool = top.enter_context(tc.tile_pool(name="otpool", bufs=8))

        # ---- projections -------------------------------------------------
        with tc.tile_pool(name="ystream", bufs=1) as ypool:
            yt = ypool.tile([128, KC, M], f32r, name="yt")
            for c in range(KC):
                nc.scalar.dma_start(yt[:, c, :], yT[ts(c, 128), :])

            # Q^T: stream xT, all 8 psum banks accumulate over k-chunks
            with tc.tile_pool(name="xstream", bufs=3) as xpool, \
                 tc.tile_pool(name="qpsum", bufs=1, space="PSUM") as qpsum:
                qps = [qpsum.tile([128, 512], f32, name=f"qps{t}")
                       for t in range(8)]
                for c in range(KC):
                    xt = xpool.tile([128, N], f32r, name="xt")
                    nc.sync.dma_start(xt, xT[ts(c, 128), :])
                    if c == 0:
                        nc.sync.dma_start(wq_sb[:, 1:KC, :], wq_r[:, 1:KC, :])
                    for jt in range(2):
                        for ic in range(4):
                            nc.tensor.matmul(
                                qps[jt * 4 + ic],
                                wq_sb[:, c, ts(jt, 128)],
                                xt[:, ts(ic, 512)],
                                start=(c == 0), stop=(c == KC - 1))
                for jt in range(2):
                    for ic in range(4):
                        nc.vector.tensor_copy(QT[jt][:, ts(ic, 512)],
                                              qps[jt * 4 + ic])

            # K^T from resident yT
            with tc.tile_pool(name="kpsum", bufs=1, space="PSUM") as kpsum:
                kps = [kpsum.tile([128, 512], f32, name=f"kps{t}")
                       for t in range(8)]
                for c in range(KC):
                    for jt in range(2):
                        for ic in range(4):
                            nc.tensor.matmul(
                                kps[jt * 4 + ic],
                                wk_sb[:, c, ts(jt, 128)],
                                yt[:, c, ts(ic, 512)],
                                start=(c == 0), stop=(c == KC - 1))
                for jt in range(2):
                    for ic in range(4):
                        nc.vector.tensor_copy(KT[jt][:, ts(ic, 512)],
                                              kps[jt * 4 + ic])

            # V natural layout [n, j], k-chunk inner
            with tc.tile_pool(name="vpsum", bufs=4, space="PSUM") as vpsum:
                for n in range(NT):
                    vp = vpsum.tile([128, J], f32, name="vp")
                    for c in range(KC):
                        nc.tensor.matmul(
                            vp,
                            yt[:, c, ts(n, 128)],
                            wv_sb[:, c, :],
                            start=(c == 0), stop=(c == KC - 1))
                    nc.vector.tensor_copy(
                        V_sb[:, n, :, 0:HEAD_DIM],
                        vp.rearrange("p (h d) -> p h d", h=GH))

        # ---- attention ---------------------------------------------------
        with tc.tile_pool(name="ppool", bufs=4) as ppool, \
             tc.tile_pool(name="rpool", bufs=4) as rpool, \
             tc.tile_pool(name="rbpool", bufs=3) as rbpool, \
             tc.tile_pool(name="rdram", bufs=3, space="DRAM") as rdram, \
             tc.tile_pool(name="spsum", bufs=2, space="PSUM") as spsum, \
             tc.tile_pool(name="opsum", bufs=1, space="PSUM") as opsum:
            for ib in range(IB):
                i_sl = ts(ib, IBS)
                oacc = {}
                for pr in range(2):
                    for lh in range(2):
                        oacc[(pr, lh)] = opsum.tile(
                            [HEAD_DIM + 1, IBS], f32, name=f"o{pr}{lh}")
                pts = {}
                for n in range(NT):
                    # S^T + exp for both head pairs of this n; the two
                    # pairs form independent ACT chains that hide each
                    # other's semaphore latency
                    for pr in range(2):
                        sp = spsum.tile([128, 2 * IBS], f32, name="sp")
                        tp_lo = dict(tile_position=(0, 0)) if PACK_S else {}
                        tp_hi = dict(tile_position=(64, 0)) if PACK_S else {}
                        nc.tensor.matmul(
                            sp[:, 0:IBS],
                            KT[pr][0:64, ts(n, 128)],
                            QT[pr][0:64, i_sl],
                            start=True, stop=True, **tp_lo)
                        nc.tensor.matmul(
                            sp[:, IBS:2 * IBS],
                            KT[pr][64:128, ts(n, 128)],
                            QT[pr][64:128, i_sl],
                            start=True, stop=True, **tp_hi)
                        pt = ppool.tile([128, 2 * IBS], f32r, name="pt")
                        nc.scalar.activation(pt, sp, Exp, bias=0.0,
                                             scale=float(SCALE))
                        pts[(pr, n)] = pt
                    # O^T accumulation for the previous n (software
                    # pipeline: keeps PE from blocking on the fresh exp)
                    if n > 0:
                        for pr in range(2):
                            pt = pts.pop((pr, n - 1))
                            for lh in range(2):
                                nc.tensor.matmul(
                                    oacc[(pr, lh)],
                                    V_sb[:, n - 1, 2 * pr + lh, :],
                                    pt[:, lh * IBS:(lh + 1) * IBS],
                                    start=(n - 1 == 0), stop=False)
                for pr in range(2):
                    pt = pts.pop((pr, NT - 1))
                    for lh in range(2):
                        nc.tensor.matmul(
                            oacc[(pr, lh)],
                            V_sb[:, NT - 1, 2 * pr + lh, :],
                            pt[:, lh * IBS:(lh + 1) * IBS],
                            start=False, stop=True)
                # PSUM-releasing evacuations for both pairs first (the
                # next round's O matmuls wait on these slots), then the
                # off-critical-path normalize chains.
                blk = {}
                for pr in range(2):
                    o_lo, o_hi = oacc[(pr, 0)], oacc[(pr, 1)]
                    ot = otpool.tile([128, IBS], f32r, name="ot")
                    nc.vector.tensor_copy(ot[0:64, :], o_lo[0:64, :])
                    nc.vector.tensor_copy(ot[64:128, :], o_hi[0:64, :])
                    rs_lo = rpool.tile([1, IBS], f32, name="rslo")
                    rs_hi = rpool.tile([1, IBS], f32, name="rshi")
                    nc.vector.tensor_copy(rs_lo, o_lo[64:65, :])
                    nc.vector.tensor_copy(rs_hi, o_hi[64:65, :])
                    blk[pr] = (ot, rs_lo, rs_hi)
                for pr in range(2):
                    ot, rs_lo, rs_hi = blk[pr]
                    rd = rdram.tile([2, IBS], f32, name="rd")
                    nc.sync.dma_start(rd[0:1, :], rs_lo)
                    nc.sync.dma_start(rd[1:2, :], rs_hi)
                    rb = rbpool.tile([128, IBS], f32, name="rb")
                    nc.sync.dma_start(rb[0:64, :],
                                      rd[0:1, :].partition_broadcast(64))
                    nc.sync.dma_start(rb[64:128, :],
                                      rd[1:2, :].partition_broadcast(64))
                    rb2 = rbpool.tile([128, IBS], f32, name="rb2")
                    nc.vector.reciprocal(rb2, rb)
                    nc.vector.tensor_mul(ot, ot.bitcast(f32), rb2)
                    ot_tiles[(ib, pr)] = ot

        # ---- output projection ------------------------------------------
        with tc.tile_pool(name="obpool", bufs=4) as obpool, \
             tc.tile_pool(name="oppsum", bufs=4, space="PSUM") as oppsum:
            for ib in range(IB):
                for icr in range(IBS // 128):
                    for cc in range(DIM // 512):
                        op = oppsum.tile([128, 512], f32, name="op")
                        for jt in range(2):
                            nc.tensor.matmul(
                                op,
                                ot_tiles[(ib, jt)][:, ts(icr, 128)],
                                wp_sb[:, jt, ts(cc, 512)],
                                start=(jt == 0), stop=(jt == 1))
                        ob = obpool.tile([128, 512], f32, name="ob")
                        nc.vector.tensor_copy(ob, op)
                        nc.sync.dma_start(
                            out[ds(ib * IBS + icr * 128, 128), ts(cc, 512)],
                            ob)

    nc.compile()
    return nc


def _get_nc():
    global _NC
    if _NC is None:
        _NC = _build()
    return _NC


def _shard_inputs(x, y, Wq, Wk, Wv, Wp):
    x = np.asarray(x, np.float32)
    y = np.asarray(y, np.float32)
    Wq = np.asarray(Wq, np.float32)
    Wk = np.asarray(Wk, np.float32)
    Wv = np.asarray(Wv, np.float32)
    Wp = np.asarray(Wp, np.float32)
    xT = [np.ascontiguousarray(x[b].T) for b in range(B)]
    yT = [np.ascontiguousarray(y[b].T) for b in range(B)]
    in_maps = []
    for c in range(NCORES):
        b, g = divmod(c, NCORES // B)
        sl = slice(g * J, (g + 1) * J)
        in_maps.append({
            "xT": xT[b],
            "yT": yT[b],
            "wq": np.ascontiguousarray(Wq[:, sl]),
            "wk": np.ascontiguousarray(Wk[:, sl]),
            "wv": np.ascontiguousarray(Wv[:, sl]),
            "wp": np.ascontiguousarray(Wp[sl, :]),
        })
    return in_maps


def run(inputs, trace=False, **spmd_kwargs):
    from concourse.bass_utils import run_bass_kernel_spmd
    nc = _get_nc()
    in_maps = _shard_inputs(inputs["x"], inputs["y"], inputs["Wq"],
                            inputs["Wk"], inputs["Wv"], inputs["Wp"])
    res = run_bass_kernel_spmd(nc, in_maps, core_ids=list(range(NCORES)),
                               trace=trace, **spmd_kwargs)
    bp = np.asarray(inputs["bp"], np.float32)
    gpb = NCORES // B
    full = np.empty((B, N, DIM), np.float32)
    for b in range(B):
        acc = res.results[b * gpb]["out"].astype(np.float32)
        for g in range(1, gpb):
            acc = acc + res.results[b * gpb + g]["out"]
        full[b] = acc + bp
    return full, res


def kernel(**inputs):
    out, _ = run(inputs, trace=False)
    return out


# revision 27
# speedup vs baseline: 1.0195x; 1.0019x over previous
"""Cross-attention kernel for 8 Trainium2 NeuronCores.

Sharding: data-parallel over batch (B=2) x tensor-parallel over heads
(16 heads -> 4 groups of 4 heads).  Core c handles batch c//4, head
group c%4.  Each core computes, for its 4 heads:
    Q^T = Wq_g^T x_b^T        [256, 2048]   (d-on-partitions layout)
    K^T = Wk_g^T y_b^T        [256, 2048]
    V   = y_b Wv_g            [2048, 256]   (n-on-partitions layout)
    S^T_h = K_h Q_h^T / 8; P^T = exp(S^T)
    O^T_h (+row sums via a ones-column in V) = [V_h|1]^T P^T
    partial = (O^T/rowsum)^T Wp_g           [2048, 1024]
The 4 partials per batch are summed on the host and bp is added.

Matmuls run as float32r (full-rate fp32 on the PE at moving-dim>=256).
Head pairs share the PE array via tile_position row packing for the
S^T matmuls; two query blocks are interleaved per head pair so the
exp (ScalarE) chain of one block hides the semaphore latency of the
other.
"""

import numpy as np

B = 2
N = 2048          # query sequence length
M = 2048          # key sequence length
DIM = 1024
HEAD_DIM = 64
SCALE = HEAD_DIM ** -0.5
NCORES = 8
GH = 4            # heads per core
J = GH * HEAD_DIM # 256 projected columns per core
KC = DIM // 128   # 8 contraction chunks
NT = M // 128     # 16 key tiles
IBS = 512         # i-block size
IB = N // IBS     # 4 i-blocks

PACK_S = True

_NC = None


def _build():
    from contextlib import ExitStack

    import concourse.bass as bass
    import concourse.tile as tile
    from concourse import bacc, mybir
    from concourse.bass import ts, ds
    from concourse.masks import make_identity

    f32 = mybir.dt.float32
    f32r = mybir.dt.float32r
    Exp = mybir.ActivationFunctionType.Exp

    nc = bacc.Bacc("TRN2", target_bir_lowering=False, debug=False,
                   num_devices=NCORES)
    xT = nc.dram_tensor("xT", [DIM, N], f32r, kind="ExternalInput").ap()
    yT = nc.dram_tensor("yT", [DIM, M], f32r, kind="ExternalInput").ap()
    wq = nc.dram_tensor("wq", [DIM, J], f32r, kind="ExternalInput").ap()
    wk = nc.dram_tensor("wk", [DIM, J], f32r, kind="ExternalInput").ap()
    wv = nc.dram_tensor("wv", [DIM, J], f32r, kind="ExternalInput").ap()
    wp = nc.dram_tensor("wp", [J, DIM], f32r, kind="ExternalInput").ap()
    out = nc.dram_tensor("out", [N, DIM], f32, kind="ExternalOutput").ap()

    with tile.TileContext(nc) as tc, ExitStack() as top:
        wpool = top.enter_context(tc.tile_pool(name="weights", bufs=1))
        wq_sb = wpool.tile([128, KC, J], f32r, name="wq_sb")
        wk_sb = wpool.tile([128, KC, J], f32r, name="wk_sb")
        wv_sb = wpool.tile([128, KC, J], f32r, name="wv_sb")
        wp_sb = wpool.tile([128, 2, DIM], f32r, name="wp_sb")
        # wq + x stream on the SP (sync) HWDGE queue; everything else on
        # the Activation HWDGE queue so Q's inputs aren't stuck behind 12MB
        wq_r = wq.rearrange("(c p) j -> p c j", p=128)
        nc.sync.dma_start(wq_sb[:, 0, :], wq_r[:, 0, :])
        nc.scalar.dma_start(wk_sb, wk.rearrange("(c p) j -> p c j", p=128))
        nc.scalar.dma_start(wv_sb, wv.rearrange("(c p) j -> p c j", p=128))
        nc.scalar.dma_start(wp_sb, wp.rearrange("(t p) c -> p t c", p=128))

        big = top.enter_context(tc.tile_pool(name="big", bufs=1))
        QT = [big.tile([128, N], f32r, name=f"qt{t}") for t in range(2)]
        KT = [big.tile([128, M], f32r, name=f"kt{t}") for t in range(2)]
        V_sb = big.tile([128, NT, GH, HEAD_DIM + 1], f32r, name="v_sb")
        # ones column for the row-sum trick: fill everything with 1.0 once;
        # the V evacuation below overwrites columns 0..63 of each (n, h)
        nc.vector.memset(V_sb.bitcast(f32), 1.0)

        ot_tiles = {}
        otpool = top.enter_context(tc.tile_pool(name="otpool", bufs=8))

        # ---- projections -------------------------------------------------
        with tc.tile_pool(name="ystream", bufs=1) as ypool:
            yt = ypool.tile([128, KC, M], f32r, name="yt")
            for c in range(KC):
                nc.scalar.dma_start(yt[:, c, :], yT[ts(c, 128), :])

            # Q^T: stream xT, all 8 psum banks accumulate over k-chunks
            with tc.tile_pool(name="xstream", bufs=3) as xpool, \
                 tc.tile_pool(name="qpsum", bufs=1, space="PSUM") as qpsum:
                qps = [qpsum.tile([128, 512], f32, name=f"qps{t}")
                       for t in range(8)]
                for c in range(KC):
                    xt = xpool.tile([128, N], f32r, name="xt")
                    nc.sync.dma_start(xt, xT[ts(c, 128), :])
                    if c == 0:
                        nc.sync.dma_start(wq_sb[:, 1:KC, :], wq_r[:, 1:KC, :])
                    for jt in range(2):
                        for ic in range(4):
                            nc.tensor.matmul(
                                qps[jt * 4 + ic],
                                wq_sb[:, c, ts(jt, 128)],
                                xt[:, ts(ic, 512)],
                                start=(c == 0), stop=(c == KC - 1))
                for jt in range(2):
                    for ic in range(4):
                        nc.vector.tensor_copy(QT[jt][:, ts(ic, 512)],
                                              qps[jt * 4 + ic])

            # K^T from resident yT
            with tc.tile_pool(name="kpsum", bufs=1, space="PSUM") as kpsum:
                kps = [kpsum.tile([128, 512], f32, name=f"kps{t}")
                       for t in range(8)]
                for c in range(KC):
                    for jt in range(2):
                        for ic in range(4):
                            nc.tensor.matmul(
                                kps[jt * 4 + ic],
                                wk_sb[:, c, ts(jt, 128)],
                                yt[:, c, ts(ic, 512)],
                                start=(c == 0), stop=(c == KC - 1))
                for jt in range(2):
                    for ic in range(4):
                        nc.vector.tensor_copy(KT[jt][:, ts(ic, 512)],
                                              kps[jt * 4 + ic])

            # V natural layout [n, j], k-chunk inner
            with tc.tile_pool(name="vpsum", bufs=4, space="PSUM") as vpsum:
                for n in range(NT):
                    vp = vpsum.tile([128, J], f32, name="vp")
                    for c in range(KC):
                        nc.tensor.matmul(
                            vp,
                            yt[:, c, ts(n, 128)],
                            wv_sb[:, c, :],
                            start=(c == 0), stop=(c == KC - 1))
                    nc.vector.tensor_copy(
                        V_sb[:, n, :, 0:HEAD_DIM],
                        vp.rearrange("p (h d) -> p h d", h=GH))

        # ---- attention ---------------------------------------------------
        # SBUF pools stay alive to the end of the kernel so the output
        # projection pools don't inherit released-zone drain dependencies
        ppool = top.enter_context(tc.tile_pool(name="ppool", bufs=4))
        rpool = top.enter_context(tc.tile_pool(name="rpool", bufs=4))
        rbpool = top.enter_context(tc.tile_pool(name="rbpool", bufs=3))
        rdram = top.enter_context(tc.tile_pool(name="rdram", bufs=3,
                                               space="DRAM"))
        with tc.tile_pool(name="spsum", bufs=2, space="PSUM") as spsum, \
             tc.tile_pool(name="opsum", bufs=1, space="PSUM") as opsum:
            for ib in range(IB):
                i_sl = ts(ib, IBS)
                oacc = {}
                for pr in range(2):
                    for lh in range(2):
                        oacc[(pr, lh)] = opsum.tile(
                            [HEAD_DIM + 1, IBS], f32, name=f"o{pr}{lh}")
                pts = {}
                for n in range(NT):
                    # S^T + exp for both head pairs of this n; the two
                    # pairs form independent ACT chains that hide each
                    # other's semaphore latency
                    for pr in range(2):
                        sp = spsum.tile([128, 2 * IBS], f32, name="sp")
                        tp_lo = dict(tile_position=(0, 0)) if PACK_S else {}
                        tp_hi = dict(tile_position=(64, 0)) if PACK_S else {}
                        nc.tensor.matmul(
                            sp[:, 0:IBS],
                            KT[pr][0:64, ts(n, 128)],
                            QT[pr][0:64, i_sl],
                            start=True, stop=True, **tp_lo)
                        nc.tensor.matmul(
                            sp[:, IBS:2 * IBS],
                            KT[pr][64:128, ts(n, 128)],
                            QT[pr][64:128, i_sl],
                            start=True, stop=True, **tp_hi)
                        pt = ppool.tile([128, 2 * IBS], f32r, name="pt")
                        nc.scalar.activation(pt, sp, Exp, bias=0.0,
                                             scale=float(SCALE))
                        pts[(pr, n)] = pt
                    # O^T accumulation for the previous n (software
                    # pipeline: keeps PE from blocking on the fresh exp)
                    if n > 0:
                        for pr in range(2):
                            pt = pts.pop((pr, n - 1))
                            for lh in range(2):
                                nc.tensor.matmul(
                                    oacc[(pr, lh)],
                                    V_sb[:, n - 1, 2 * pr + lh, :],
                                    pt[:, lh * IBS:(lh + 1) * IBS],
                                    start=(n - 1 == 0), stop=False)
                for pr in range(2):
                    pt = pts.pop((pr, NT - 1))
                    for lh in range(2):
                        nc.tensor.matmul(
                            oacc[(pr, lh)],
                            V_sb[:, NT - 1, 2 * pr + lh, :],
                            pt[:, lh * IBS:(lh + 1) * IBS],
                            start=False, stop=True)
                # PSUM-releasing evacuations for both pairs first (the
                # next round's O matmuls wait on these slots), then the
                # off-critical-path normalize chains.
                blk = {}
                for pr in range(2):
                    o_lo, o_hi = oacc[(pr, 0)], oacc[(pr, 1)]
                    ot = otpool.tile([128, IBS], f32r, name="ot")
                    nc.vector.tensor_copy(ot[0:64, :], o_lo[0:64, :])
                    nc.vector.tensor_copy(ot[64:128, :], o_hi[0:64, :])
                    rs_lo = rpool.tile([1, IBS], f32, name="rslo")
                    rs_hi = rpool.tile([1, IBS], f32, name="rshi")
                    nc.vector.tensor_copy(rs_lo, o_lo[64:65, :])
                    nc.vector.tensor_copy(rs_hi, o_hi[64:65, :])
                    blk[pr] = (ot, rs_lo, rs_hi)
                for pr in range(2):
                    ot, rs_lo, rs_hi = blk[pr]
                    rd = rdram.tile([2, IBS], f32, name="rd")
                    nc.sync.dma_start(rd[0:1, :], rs_lo)
                    nc.sync.dma_start(rd[1:2, :], rs_hi)
                    rb = rbpool.tile([128, IBS], f32, name="rb")
                    nc.sync.dma_start(rb[0:64, :],
                                      rd[0:1, :].partition_broadcast(64))
                    nc.sync.dma_start(rb[64:128, :],
                                      rd[1:2, :].partition_broadcast(64))
                    rb2 = rbpool.tile([128, IBS], f32, name="rb2")
                    nc.vector.reciprocal(rb2, rb)
                    nc.vector.tensor_mul(ot, ot.bitcast(f32), rb2)
                    ot_tiles[(ib, pr)] = ot

        # ---- output projection ------------------------------------------
        obpool = top.enter_context(tc.tile_pool(name="obpool", bufs=4))
        with tc.tile_pool(name="oppsum", bufs=4, space="PSUM") as oppsum:
            for ib in range(IB):
                for icr in range(IBS // 128):
                    for cc in range(DIM // 512):
                        op = oppsum.tile([128, 512], f32, name="op")
                        for jt in range(2):
                            nc.tensor.matmul(
                                op,
                                ot_tiles[(ib, jt)][:, ts(icr, 128)],
                                wp_sb[:, jt, ts(cc, 512)],
                                start=(jt == 0), stop=(jt == 1))
                        ob = obpool.tile([128, 512], f32, name="ob")
                        nc.vector.tensor_copy(ob, op)
                        nc.sync.dma_start(
                            out[ds(ib * IBS + icr * 128, 128), ts(cc, 512)],
                            ob)

    nc.compile()
    return nc


def _get_nc():
    global _NC
    if _NC is None:
        _NC = _build()
    return _NC


def _shard_inputs(x, y, Wq, Wk, Wv, Wp):
    x = np.asarray(x, np.float32)
    y = np.asarray(y, np.float32)
    Wq = np.asarray(Wq, np.float32)
    Wk = np.asarray(Wk, np.float32)
    Wv = np.asarray(Wv, np.float32)
    Wp = np.asarray(Wp, np.float32)
    xT = [np.ascontiguousarray(x[b].T) for b in range(B)]
    yT = [np.ascontiguousarray(y[b].T) for b in range(B)]
    in_maps = []
    for c in range(NCORES):
        b, g = divmod(c, NCORES // B)
        sl = slice(g * J, (g + 1) * J)
        in_maps.append({
            "xT": xT[b],
            "yT": yT[b],
            "wq": np.ascontiguousarray(Wq[:, sl]),
            "wk": np.ascontiguousarray(Wk[:, sl]),
            "wv": np.ascontiguousarray(Wv[:, sl]),
            "wp": np.ascontiguousarray(Wp[sl, :]),
        })
    return in_maps


def run(inputs, trace=False, **spmd_kwargs):
    from concourse.bass_utils import run_bass_kernel_spmd
    nc = _get_nc()
    in_maps = _shard_inputs(inputs["x"], inputs["y"], inputs["Wq"],
                            inputs["Wk"], inputs["Wv"], inputs["Wp"])
    res = run_bass_kernel_spmd(nc, in_maps, core_ids=list(range(NCORES)),
                               trace=trace, **spmd_kwargs)
    bp = np.asarray(inputs["bp"], np.float32)
    gpb = NCORES // B
    full = np.empty((B, N, DIM), np.float32)
    for b in range(B):
        acc = res.results[b * gpb]["out"].astype(np.float32)
        for g in range(1, gpb):
            acc = acc + res.results[b * gpb + g]["out"]
        full[b] = acc + bp
    return full, res


def kernel(**inputs):
    out, _ = run(inputs, trace=False)
    return out
